# revision 42
# baseline (speedup 1.0000x reference)
"""Distributed Bass kernel for nn_Attention (B=2, N=2048, D=1024, H=16, DH=64) on 8 trn2 cores.

Sharding: data-parallel over batch (cores 0-3 -> b=0, 4-7 -> b=1), tensor-parallel
over heads (4 heads / 256 inner features per core).  v3 design:
  all matmuls bf16 (weights/x pre-cast+rearranged on host, fp32 PSUM accumulate);
  q/k projections + rope first (rope combine split DVE / ACT-copy+GPSIMD-add),
  ssq AllReduce (4-core groups) overlapped with v projections;
  k-side RMSNorm scale folded into the softmax exp *scale* (per-key partition
  scale AP) so kT needs no normalization pass at all;
  softmax exp split across two engine lanes: ACT table exp and a DVE
  Schraudolph exp (affine int16 + bf16 bitcast, ~38% of tiles);
  softmax denominator via ones-row in v_aug, fast-approx reciprocal;
  per-head-pair AllGather over the 4-core batch group (non-shared output),
  output projection in two rounds (even feature chunks overlap the final
  AllGather, odd chunks after it).
Host assembles the (2, 2048, 1024) output from the 8 (512, 1024) shards.
"""
import os
import sys

for _p in ("/opt/trn_rl_repo", "/root/.axon_site/_ro/trn_rl_repo"):
    if os.path.isdir(_p) and _p not in sys.path:
        sys.path.insert(0, _p)

import numpy as np
import ml_dtypes
import concourse.bass as bass
import concourse.mybir as mybir
import concourse.tile as tile
from concourse import bacc
from concourse.bass_utils import run_bass_kernel_spmd

dt = mybir.dt
AF = mybir.ActivationFunctionType
ALU = mybir.AluOpType
F32, BF16, I32, I16 = dt.float32, dt.bfloat16, dt.int32, dt.int16
BF = ml_dtypes.bfloat16

B, N, D = 2, 2048, 1024
H, DH = 16, 64
HPC = 4            # heads per core
FPC = HPC * DH     # 256 inner features per core
KC = D // 128      # 8 contraction chunks
FC = FPC // 128    # 2 feature chunks per core
NJ = N // 512      # 4 quarter chunks
NT = N // 128      # 16 m-tiles
EPS = 1e-6
CORES = 8
GROUPS4 = [[0, 1, 2, 3], [4, 5, 6, 7]]
JORDER = [2, 3, 0, 1]  # n-half 1 first: its ssq AllReduce fires early

L2E = float(np.log2(np.e))
SCH_SCALE = 128.0 * L2E            # Schraudolph bf16 exp scale
SCH_BIAS = 127.0 * 128.0 - 5.61    # fitted for RNE rounding, max rel ~3.3%

_CACHED_NC = None


def build(dbg=False):
    nc = bacc.Bacc("TRN2", target_bir_lowering=False, debug=False, num_devices=CORES)

    xT = nc.dram_tensor("xT", [128, KC, N], BF16, kind="ExternalInput")
    wq_d = nc.dram_tensor("wq", [128, KC, FPC], BF16, kind="ExternalInput")
    wk_d = nc.dram_tensor("wk", [128, KC, FPC], BF16, kind="ExternalInput")
    wv_d = nc.dram_tensor("wv", [128, KC, FPC], BF16, kind="ExternalInput")
    wo_d = nc.dram_tensor("wo", [128, KC, D], BF16, kind="ExternalInput")
    bo_d = nc.dram_tensor("bo", [128, D], F32, kind="ExternalInput")
    wqc_d = nc.dram_tensor("wqc", [128, FC], BF16, kind="ExternalInput")
    wkc_d = nc.dram_tensor("wkc", [128, FC], BF16, kind="ExternalInput")
    cos_d = nc.dram_tensor("cos_t", [128, N], BF16, kind="ExternalInput")
    sin_d = nc.dram_tensor("sin_t", [128, N], BF16, kind="ExternalInput")
    psw_d = nc.dram_tensor("psw_t", [128, 128], BF16, kind="ExternalInput")
    gidx_d = nc.dram_tensor("gidx", [128, KC], I32, kind="ExternalInput")
    out_d = nc.dram_tensor("out", [512, D], F32, kind="ExternalOutput")

    if dbg:
        dbg_qn = nc.dram_tensor("dbg_qn", [128, FC, N], BF16, kind="ExternalOutput")
        dbg_kt = nc.dram_tensor("dbg_kt", [128, FC, N], BF16, kind="ExternalOutput")
        dbg_rk = nc.dram_tensor("dbg_rk", [128, NT], F32, kind="ExternalOutput")
        dbg_osb = nc.dram_tensor("dbg_osb", [128, FC, N], BF16, kind="ExternalOutput")
        dbg_og = nc.dram_tensor("dbg_og", [128, KC, 512], BF16, kind="ExternalOutput")

    # collective bounce buffers
    rk_bounce = nc.dram_tensor("rk_bounce", [1, N], F32)
    ssq_in = [nc.dram_tensor(f"ssq_in{i}", [2, N // 2], F32) for i in range(2)]
    ssq_out = [nc.dram_tensor(f"ssq_out{i}", [2, N // 2], F32) for i in range(2)]
    ag_in0 = nc.dram_tensor("ag_in0", [128, N], BF16)
    ag_out0 = nc.dram_tensor("ag_out0", [4 * 128, N], BF16)
    # pair 1 gathers per query half; halves are contiguous row blocks
    ag_in1 = nc.dram_tensor("ag_in1", [2 * 128, N // 2], BF16)
    ag_out1 = nc.dram_tensor("ag_out1", [2 * 4 * 128, N // 2], BF16)

    with tile.TileContext(nc) as tc:
        with tc.tile_pool(name="persist", bufs=1) as pp:
            # ---- constants ------------------------------------------------
            ones_col32 = pp.tile([128, 1], F32, tag="onesc32")
            nc.gpsimd.memset(ones_col32[:], 1.0)
            ones_col_bf = pp.tile([128, 1], BF16, tag="onescbf")
            nc.vector.tensor_copy(ones_col_bf[:], ones_col32[:])
            ones_row32 = pp.tile([1, 128], F32, tag="onesr32")
            nc.gpsimd.memset(ones_row32[:], 1.0)
            ones_row_bf = pp.tile([1, 128], BF16, tag="onesrbf")
            nc.vector.tensor_copy(ones_row_bf[:], ones_row32[:])
            # activation bias values at consumer base partitions
            eps_t = pp.tile([33, 1], F32, tag="eps")
            nc.gpsimd.memset(eps_t[:], EPS)
            bexp_t = pp.tile([33, 1], F32, tag="bexp")
            nc.gpsimd.memset(bexp_t[:], 0.0)
            nc.gpsimd.memset(bexp_t[32:33, :], -float(np.log(8.0)))

            wqc_sb = pp.tile([128, FC], BF16, tag="wqc")
            wkc_sb = pp.tile([128, FC], BF16, tag="wkc")
            gidx_sb = pp.tile([128, KC], I32, tag="gidx")

            # ---- big persistent tensors ----------------------------------
            # DMA order matters: the first q/k matmul group needs xsb j=0 and
            # wq only, so those go first on the queue
            xsb = pp.tile([128, KC, N], BF16, tag="xsb")
            wq_sb = pp.tile([128, KC, FPC], BF16, tag="wq")
            wk_sb = pp.tile([128, KC, FPC], BF16, tag="wk")
            wv_sb = pp.tile([128, KC, FPC], BF16, tag="wv")
            cos_sb = pp.tile([128, N], BF16, tag="cos")
            sin_sb = pp.tile([128, N], BF16, tag="sin")
            # PE warmup: dummy matmuls from kernel start until the first real
            # projection inputs land -- moves the HAM to K=8/8 before phase A
            # and avoids a cold first half
            wrm = pp.tile([128, 512], BF16, tag="wrm")
            nc.gpsimd.memset(wrm[:], 1.0)
            with tc.tile_pool(name="psW", bufs=1, space="PSUM") as psW:
                wps = psW.tile([128, 512], F32, tag="wps")
                for _ in range(20):
                    nc.tensor.matmul(
                        wps[:], wrm[:, 0:128], wrm[:], start=True, stop=True
                    )

            # n-half 1 first (its ssq AllReduce fires early); initial loads
            # split across the sync/scalar queues so the first matmul group's
            # inputs land in ~6us instead of ~14us
            h1sl = slice(1024, 2048)
            h0sl = slice(0, 1024)
            nc.scalar.dma_start(out=wq_sb[:], in_=wq_d[:])
            for kc in range(KC):
                if kc < 6:
                    nc.sync.dma_start(out=xsb[:, kc, h1sl], in_=xT[:, kc, h1sl])
                else:
                    nc.scalar.dma_start(out=xsb[:, kc, h1sl], in_=xT[:, kc, h1sl])
            # norm-weight columns feed the very first rope STT: keep them at
            # the front of the (otherwise free) ACT queue
            nc.scalar.dma_start(out=wqc_sb[:], in_=wqc_d[:])
            nc.scalar.dma_start(out=wkc_sb[:], in_=wkc_d[:])
            # pair-swap permutation matrix (host-precomputed):
            # psw[p, 2f+e] = 1 iff p == 2f+1-e
            psw = pp.tile([128, 128], BF16, tag="psw")
            nc.sync.dma_start(out=psw[:], in_=psw_d[:])
            nc.sync.dma_start(out=cos_sb[:, h1sl], in_=cos_d[:, h1sl])
            nc.sync.dma_start(out=sin_sb[:, h1sl], in_=sin_d[:, h1sl])
            nc.sync.dma_start(out=wk_sb[:], in_=wk_d[:])
            nc.sync.dma_start(out=xsb[:, :, h0sl], in_=xT[:, :, h0sl])
            nc.sync.dma_start(out=cos_sb[:, h0sl], in_=cos_d[:, h0sl])
            nc.sync.dma_start(out=sin_sb[:, h0sl], in_=sin_d[:, h0sl])
            nc.sync.dma_start(out=gidx_sb[:], in_=gidx_d[:])
            # big late-use weights ride the back of the sync queue: keeps the
            # ACT queue free for phase A compute and the gpsimd queue free
            # for prompt collective triggers (SWDGE issue is ~2us per DMA)
            nc.sync.dma_start(out=wv_sb[:], in_=wv_d[:])
            wo_sb = pp.tile([128, KC, D], BF16, tag="wo")
            nc.sync.dma_start(out=wo_sb[:], in_=wo_d[:])
            bo_sb = pp.tile([128, D], F32, tag="bo")
            nc.sync.dma_start(out=bo_sb[:], in_=bo_d[:])
            # PE warmup: dummy matmuls from kernel start until the first real
            # projection inputs land -- moves the HAM to K=8/8 before phase A
            # and avoids a cold first half
            wrm = pp.tile([128, 512], BF16, tag="wrm")
            nc.gpsimd.memset(wrm[:], 1.0)
            with tc.tile_pool(name="psW", bufs=1, space="PSUM") as psW:
                wps = psW.tile([128, 512], F32, tag="wps")
                for _ in range(40):
                    nc.tensor.matmul(
                        wps[:], wrm[:, 0:128], wrm[:], start=True, stop=True
                    )

            kT = pp.tile([128, FC, N], BF16, tag="kT")
            qn = pp.tile([128, FC, N], BF16, tag="qn")
            v_aug = pp.tile([128, NT, HPC, DH + 1], BF16, tag="vaug")
            nc.vector.tensor_copy(
                v_aug[:, :, :, DH : DH + 1],
                ones_col32[:].to_broadcast([128, NT, HPC, 1]),
            )
            o_sb = pp.tile([128, FC, N], BF16, tag="osb")
            og = pp.tile([128, KC, 512], BF16, tag="og")
            rinv = pp.tile([1, N], BF16, tag="rinv")
            rk0 = pp.tile([1, N], F32, tag="rk0")
            rk8T = pp.tile([128, NT], F32, tag="rk8T")
            rk8Td = pp.tile([128, NT], F32, tag="rk8Td")

            # ================= PHASE A: q/k projections + rope =============
            with (
                tc.tile_pool(name="pa", bufs=1) as pa,
                tc.tile_pool(name="pwa", bufs=4) as pwa,
            ):
                qpre = pa.tile([128, FC, N], BF16, tag="qpre")
                # row-vector stripes at base partitions 0 (q) and 32 (k)
                ssqp = pa.tile([33, N], F32, tag="ssqp")
                sq2 = pa.tile([33, N], F32, tag="sq2")
                lnv = pa.tile([33, N], F32, tag="lnv")
                # rows 1-31 are never written by the ssq path but are read by
                # the combined [33, N] ln below; keep them finite
                nc.gpsimd.memset(sq2[:], 1.0)

                mul = ALU.mult
                with (
                    tc.tile_pool(name="psA", bufs=2, space="PSUM") as psA,
                    tc.tile_pool(name="psSw", bufs=1, space="PSUM") as psSw,
                    tc.tile_pool(name="psS", bufs=1, space="PSUM") as psS,
                ):
                  # 1024-wide tiles per (n-half, q/k, feature chunk): fewer,
                  # fatter elementwise ops keep the DVE/ACT queues short
                  for i in (1, 0):
                    hsl = slice(i * 1024, (i + 1) * 1024)
                    for ti, (w_sb, wcol, dest) in enumerate((
                        (wq_sb, wqc_sb, qpre),
                        (wk_sb, wkc_sb, kT),
                    )):
                        ssq_ps = psS.tile([1, 1024], F32, tag="ssq", name=f"ssq{i}{ti}")
                        for fc in range(FC):
                            fsl = slice(fc * 128, (fc + 1) * 128)
                            prj = psA.tile(
                                [128, 1024], F32, tag="proj", name=f"prj{i}{ti}{fc}"
                            )
                            for nh in range(2):
                                nsl = slice(nh * 512, nh * 512 + 512)
                                jsl = slice(i * 1024 + nh * 512, i * 1024 + nh * 512 + 512)
                                for kc in range(KC):
                                    nc.tensor.matmul(
                                        prj[:, nsl],
                                        w_sb[:, kc, fsl],
                                        xsb[:, kc, jsl],
                                        start=(kc == 0),
                                        stop=(kc == KC - 1),
                                    )
                            # sum-of-squares partial on ACT
                            q2 = pwa.tile([128, 1024], BF16, tag="q2", name=f"q2_{i}{ti}{fc}")
                            nc.scalar.activation(q2[:], prj[:], AF.Square)
                            for nh in range(2):
                                nsl = slice(nh * 512, nh * 512 + 512)
                                nc.tensor.matmul(
                                    ssq_ps[:, nsl],
                                    ones_col_bf[:],
                                    q2[:, nsl],
                                    start=(fc == 0),
                                    stop=(fc == FC - 1),
                                )
                            # rope with norm weight folded in; reads prj PSUM
                            tcos = pwa.tile([128, 1024], BF16, tag="tcos", name=f"tc{i}{ti}{fc}")
                            nc.vector.scalar_tensor_tensor(
                                tcos[:], prj[:], wcol[:, fc : fc + 1], cos_sb[:, hsl],
                                op0=mul, op1=mul,
                            )
                            tsin = pwa.tile([128, 1024], BF16, tag="tsin", name=f"ts{i}{ti}{fc}")
                            nc.vector.scalar_tensor_tensor(
                                tsin[:], prj[:], wcol[:, fc : fc + 1], sin_sb[:, hsl],
                                op0=mul, op1=mul,
                            )
                            swp = psSw.tile([128, 1024], F32, tag="swp", name=f"sw{i}{ti}{fc}")
                            for nh in range(2):
                                nsl = slice(nh * 512, nh * 512 + 512)
                                nc.tensor.matmul(
                                    swp[:, nsl], psw[:], tsin[:, nsl],
                                    start=True, stop=True,
                                )
                            # rope combine on DVE (gpsimd must stay free so
                            # the ssq AllReduce triggers fire promptly)
                            nc.vector.tensor_add(dest[:, fc, hsl], tcos[:], swp[:])
                        nc.scalar.activation(
                            ssqp[32 * ti : 32 * ti + 1, hsl], ssq_ps[:], AF.Copy
                        )
                    # fire this n-half's ssq AllReduce; bounce DMAs ride the
                    # near-empty ACT queue so the trigger is prompt
                    nc.scalar.dma_start(out=ssq_in[i][0:1, :], in_=ssqp[0:1, hsl])
                    nc.scalar.dma_start(out=ssq_in[i][1:2, :], in_=ssqp[32:33, hsl])
                    nc.gpsimd.collective_compute(
                        "AllReduce",
                        ALU.add,
                        replica_groups=GROUPS4,
                        ins=[ssq_in[i][:]],
                        outs=[ssq_out[i][:]],
                    )
                    nc.sync.dma_start(out=sq2[0:1, hsl], in_=ssq_out[i][0:1, :])
                    nc.sync.dma_start(out=sq2[32:33, hsl], in_=ssq_out[i][1:2, :])

                # ---- v projections part 1 (overlap the AllReduce tail) ----
                with (
                    tc.tile_pool(name="psV", bufs=4, space="PSUM") as psV,
                    tc.tile_pool(name="psBC", bufs=2, space="PSUM") as psBC,
                ):
                    def v_proj(nt):
                        nsl = slice((nt % 8) * 128, (nt % 8 + 1) * 128)
                        jsl = slice((nt // 8) * 1024, (nt // 8) * 1024 + 1024)
                        vps = psV.tile([128, FPC], F32, tag="v", name=f"v{nt}")
                        for kc in range(KC):
                            nc.tensor.matmul(
                                vps[:],
                                xsb[:, kc, jsl][:, nsl],
                                wv_sb[:, kc, :],
                                start=(kc == 0),
                                stop=(kc == KC - 1),
                            )
                        nc.scalar.activation(
                            v_aug[:, nt, :, 0:DH],
                            vps[:].rearrange("p (h f) -> p h f", f=DH),
                            AF.Copy,
                        )

                    for nt in range(12):
                        v_proj(nt)

                    # rq = rsqrt(ssq/D + eps) = exp(-0.5*ln(ssq/D + eps))
                    # rk8 = rq_k / 8      (score scale folded in, bias=-ln 8)
                    for i in (1, 0):
                        hsl = slice(i * 1024, (i + 1) * 1024)
                        nc.scalar.activation(
                            lnv[:, hsl], sq2[:, hsl], AF.Ln, scale=1.0 / D,
                            bias=eps_t[:],
                        )
                        nc.scalar.activation(
                            rinv[0:1, hsl], lnv[0:1, hsl], AF.Exp, scale=-0.5,
                        )
                        nc.scalar.activation(
                            rk0[0:1, hsl], lnv[32:33, hsl], AF.Exp, scale=-0.5,
                            bias=bexp_t[32:33, :],
                        )

                    # qn = qpre * broadcast(rq); k needs no scaling (rk8 is
                    # folded into the per-key exp scale APs below)
                    bq = {}
                    for i in (1, 0):
                        bq[i] = psBC.tile([128, 1024], F32, tag="bq", name=f"bq{i}")
                        for q2_ in range(2):
                            q2sl = slice(q2_ * 512, q2_ * 512 + 512)
                            hq2 = slice(i * 1024 + q2_ * 512, i * 1024 + q2_ * 512 + 512)
                            nc.tensor.matmul(
                                bq[i][:, q2sl], ones_row_bf[:], rinv[0:1, hq2],
                                start=True, stop=True,
                            )

                    for nt in range(12, NT):
                        v_proj(nt)

                    for i in (1, 0):
                        hsl = slice(i * 1024, (i + 1) * 1024)
                        for fc in range(FC):
                            nc.vector.tensor_mul(
                                qn[:, fc, hsl], qpre[:, fc, hsl], bq[i][:]
                            )

                    # per-key exp scales: rk8T[p, mt] = rk0[mt*128+p],
                    # rk8Td = rk8T * 128*log2(e)  (Schraudolph lane).
                    # Transposed via a DRAM bounce (both DMAs on the same
                    # in-order queue) -- a cross-partition rearrange read of
                    # SBUF is not dependency-tracked reliably.
                    nc.sync.dma_start(out=rk_bounce[:], in_=rk0[:])
                    nc.sync.dma_start(
                        out=rk8T[:],
                        in_=rk_bounce[0:1, :].rearrange("r (m p) -> (r p) m", p=128),
                    )
                    nc.vector.tensor_scalar(
                        rk8Td[:], rk8T[:], SCH_SCALE, None, op0=mul,
                    )

            # ============== PHASE B: attention, per head pair ==============
            expctr = 0
            with (
                tc.tile_pool(name="pwb", bufs=2) as pwb,
                tc.tile_pool(name="ppb", bufs=4) as ppb,
                tc.tile_pool(name="psB", bufs=2, space="PSUM") as psB,
                tc.tile_pool(name="psOV", bufs=1, space="PSUM") as psOV,
            ):
                # units interleave the two heads so consecutive ov matmuls
                # accumulate into different psum banks (back-to-back same-bank
                # accumulation serializes on the bank's read-modify-write)
                for ch in range(FC):
                    for hf in (1, 0):
                        # four independent 1-bank accumulators (head x query
                        # quarter); rows 0-63 = o, 64 = denominator
                        ov = {
                            (h2, j2): psOV.tile(
                                [DH + 1, 512], F32, tag=f"ov{h2}{j2}",
                                name=f"ov{ch}{hf}{h2}{j2}",
                            )
                            for h2 in range(2)
                            for j2 in range(2)
                        }
                        units = [(j2, mt) for j2 in range(2) for mt in range(NT)]

                        def emit_scores(k):
                            j2, mt = units[k]
                            jsl = slice(
                                hf * 1024 + j2 * 512, hf * 1024 + j2 * 512 + 512
                            )
                            s = {}
                            for h2 in range(2):
                                po = 64 * h2
                                st = psB.tile(
                                    [128, 512], F32, tag=f"s{h2}",
                                    name=f"s{ch}{hf}{j2}{mt}{h2}",
                                )
                                for mh in range(2):
                                    msl = slice(
                                        mt * 128 + mh * 64, mt * 128 + mh * 64 + 64
                                    )
                                    nc.tensor.matmul(
                                        st[mh * 64 : (mh + 1) * 64, :],
                                        kT[po : po + 64, ch, msl],
                                        qn[po : po + 64, ch, jsl],
                                        start=True,
                                        stop=True,
                                    )
                                s[h2] = st
                            return s

                        def emit_expov(k, s):
                            nonlocal expctr
                            j2, mt = units[k]
                            # DVE exps kept away from block boundaries where
                            # the DVE queue carries the evict chains
                            dve_ok = 3 <= k < 29
                            for h2 in range(2):
                                h = 2 * ch + h2
                                p = ppb.tile(
                                    [128, 512], BF16, tag=f"p{h2}",
                                    name=f"p{ch}{hf}{j2}{mt}{h2}",
                                )
                                if dve_ok and (expctr * 3) % 8 < 3:
                                    # Schraudolph exp on DVE (int16 + bitcast)
                                    nc.vector.tensor_scalar(
                                        p[:].bitcast(I16), s[h2][:],
                                        rk8Td[:, mt : mt + 1], SCH_BIAS,
                                        op0=ALU.mult, op1=ALU.add,
                                    )
                                else:
                                    nc.scalar.activation(
                                        p[:], s[h2][:], AF.Exp,
                                        scale=rk8T[:, mt : mt + 1],
                                    )
                                expctr += 1
                                nc.tensor.matmul(
                                    ov[(h2, j2)][0 : DH + 1, :],
                                    v_aug[:, mt, h, :],
                                    p[:],
                                    start=(mt == 0),
                                    stop=(mt == NT - 1),
                                )
                            if mt == NT - 1:
                                # this j2 quarter's accumulation is complete
                                # for both heads: evict while the rest runs
                                for h2 in range(2):
                                    emit_evict(j2, h2)

                        def emit_evict(j2, h2):
                            po = 64 * h2
                            ovt = ov[(h2, j2)]
                            # denominator row: psum partition 64 -> sbuf
                            # partition 0 (partition-shifting copy)
                            den = pwb.tile(
                                [1, 512], F32, tag=f"den{h2}",
                                name=f"den{ch}{hf}{j2}{h2}",
                            )
                            nc.vector.tensor_copy(den[:], ovt[DH : DH + 1, :])
                            rec = pwb.tile(
                                [1, 512], F32, tag=f"rec{h2}",
                                name=f"rec{ch}{hf}{j2}{h2}",
                            )
                            nc.vector.reciprocal_approx_fast(rec[:], den[:])
                            bc = pwb.tile(
                                [DH, 512], F32, tag=f"bc{h2}",
                                name=f"bc{ch}{hf}{j2}{h2}",
                            )
                            nc.gpsimd.partition_broadcast(bc[:], rec[:])
                            nc.vector.tensor_mul(
                                o_sb[po : po + 64, ch,
                                     hf * 1024 + j2 * 512 : hf * 1024 + j2 * 512 + 512],
                                ovt[0:DH, :],
                                bc[:],
                            )

                        prev = None
                        for k in range(len(units)):
                            cur = emit_scores(k)
                            if prev is not None:
                                emit_expov(k - 1, prev)
                            prev = cur
                        emit_expov(len(units) - 1, prev)

                        if ch == 1:
                            # pair 1 gathers per query half so only the last
                            # (half-size) AllGather is exposed at the tail
                            hsl = slice(hf * 1024, hf * 1024 + 1024)
                            rsl = slice(hf * 128, hf * 128 + 128)
                            nc.sync.dma_start(
                                out=ag_in1[rsl, :], in_=o_sb[:, 1, hsl]
                            )
                            nc.gpsimd.collective_compute(
                                "AllGather",
                                ALU.bypass,
                                replica_groups=GROUPS4,
                                ins=[ag_in1[rsl, :]],
                                outs=[ag_out1[hf * 512 : hf * 512 + 512, :]],
                            )

                    if ch == 0:
                        # pair 0: one full gather, hidden under pair 1 compute
                        nc.sync.dma_start(out=ag_in0[:], in_=o_sb[:, 0, :])
                        nc.gpsimd.collective_compute(
                            "AllGather",
                            ALU.bypass,
                            replica_groups=GROUPS4,
                            ins=[ag_in0[:]],
                            outs=[ag_out0[:]],
                        )

                # og gathers: ch0 rows are ready once AG0 lands (mid pair 1);
                # each core picks its own n-quarter via the gidx input
                for kc in range(0, KC, FC):
                    nc.gpsimd.indirect_dma_start(
                        out=og[:, kc, :],
                        out_offset=None,
                        in_=ag_out0[:].rearrange("r (j n) -> (r j) n", n=512),
                        in_offset=bass.IndirectOffsetOnAxis(
                            ap=gidx_sb[:, kc : kc + 1], axis=0
                        ),
                    )
                for kc in range(1, KC, FC):
                    nc.gpsimd.indirect_dma_start(
                        out=og[:, kc, :],
                        out_offset=None,
                        in_=ag_out1[:].rearrange("r (q n) -> (r q) n", n=512),
                        in_offset=bass.IndirectOffsetOnAxis(
                            ap=gidx_sb[:, kc : kc + 1], axis=0
                        ),
                    )

            if dbg:
                nc.gpsimd.dma_start(out=dbg_qn[:], in_=qn[:])
                nc.gpsimd.dma_start(out=dbg_kt[:], in_=kT[:])
                nc.sync.dma_start(out=dbg_rk[:], in_=rk8T[:])
                nc.sync.dma_start(out=dbg_osb[:], in_=o_sb[:])
                nc.sync.dma_start(out=dbg_og[:], in_=og[:])

            # ========= PHASE C: output projection ==========================
            # round 1: even contraction chunks (need only AG0) overlap AG1;
            # round 2: odd chunks + evictions after AG1 lands
            with (
                tc.tile_pool(name="pc", bufs=2) as pc,
                tc.tile_pool(name="psC", bufs=1, space="PSUM") as psC,
            ):
                yps = {}
                for ntl in range(4):
                    yps[ntl] = psC.tile(
                        [128, D], F32, tag=f"y{ntl}", name=f"yps{ntl}"
                    )
                    for dc in range(2):
                        dsl = slice(dc * 512, (dc + 1) * 512)
                        for i, kc in enumerate(range(0, KC, 2)):
                            nc.tensor.matmul(
                                yps[ntl][:, dsl],
                                og[:, kc, ntl * 128 : (ntl + 1) * 128],
                                wo_sb[:, kc, dsl],
                                start=(i == 0),
                                stop=False,
                            )
                for ntl in range(4):
                    for dc in range(2):
                        dsl = slice(dc * 512, (dc + 1) * 512)
                        for i, kc in enumerate(range(1, KC, 2)):
                            nc.tensor.matmul(
                                yps[ntl][:, dsl],
                                og[:, kc, ntl * 128 : (ntl + 1) * 128],
                                wo_sb[:, kc, dsl],
                                start=False,
                                stop=(i == KC // 2 - 1),
                            )
                    # bias folded into the eviction (host pre-broadcast to all
                    # 128 partitions)
                    ysb = pc.tile([128, D], F32, tag="ysb", name=f"ysb{ntl}")
                    nc.vector.tensor_add(ysb[:], yps[ntl][:], bo_sb[:])
                    nc.sync.dma_start(
                        out=out_d[ntl * 128 : (ntl + 1) * 128, :], in_=ysb[:]
                    )

    nc.compile()
    return nc


def _rope_tables():
    """cos/sin tables matching the reference's f32 angle computation.

    C[d, n] = cos(n * theta[d//2]);  Ssw[2i] = +sin, Ssw[2i+1] = -sin
    (Ssw is the swapped-operand multiplier: rope = x*C + swap(x*Ssw)).
    Tiled x2 along partitions to cover a 2-head (128-row) chunk.
    """
    i2 = np.arange(0, DH, 2, dtype=np.float32)
    theta = (1.0 / (10000.0 ** (i2 / DH))).astype(np.float32)  # (32,)
    ang = np.arange(N, dtype=np.float32)[:, None] * theta[None, :]  # (N, 32) f32
    cos = np.cos(ang.astype(np.float64)).astype(np.float32).T  # (32, N)
    sin = np.sin(ang.astype(np.float64)).astype(np.float32).T
    cos_d = np.repeat(cos, 2, axis=0)  # (64, N)
    ssw = np.repeat(sin, 2, axis=0)
    ssw[1::2, :] *= -1.0
    cos_t = np.tile(cos_d, (2, 1)).astype(np.float32)  # (128, N)
    sin_t = np.tile(ssw, (2, 1)).astype(np.float32)
    return cos_t, sin_t


def _rearr(w):
    # [D, F] -> [128, KC, F] grouping the contraction dim into 128-row chunks
    d, f = w.shape
    return np.ascontiguousarray(
        w.reshape(KC, 128, f).transpose(1, 0, 2).astype(BF)
    )


def kernel(x, Wq, Wkv, norm_q_w, norm_k_w, Wo, bo, _trace=False, _dbg=False):
    global _CACHED_NC
    x = np.asarray(x, dtype=np.float32)
    Wq = np.asarray(Wq, dtype=np.float32)
    Wkv = np.asarray(Wkv, dtype=np.float32)
    norm_q_w = np.asarray(norm_q_w, dtype=np.float32)
    norm_k_w = np.asarray(norm_k_w, dtype=np.float32)
    Wo = np.asarray(Wo, dtype=np.float32)
    bo = np.asarray(bo, dtype=np.float32)

    cos_t, sin_t = _rope_tables()
    psw_np = np.zeros((128, 128), dtype=np.float32)
    pidx = np.arange(128)
    psw_np[pidx ^ 1, pidx] = 1.0  # column 2f+e reads row 2f+1-e
    if _dbg:
        nc = build(dbg=True)
    else:
        if _CACHED_NC is None:
            _CACHED_NC = build()
        nc = _CACHED_NC

    in_maps = []
    for c in range(CORES):
        b, g = c // 4, c % 4
        fsl = slice(g * FPC, (g + 1) * FPC)
        # even kc (pair 0): ag_out0 [512, 2048] -> rows (128*c + p)*4 + g
        # odd kc (pair 1): ag_out1 [1024, 1024] half-major
        #   -> rows (512*(g//2) + 128*c + p)*2 + (g%2)
        gidx = np.empty((128, KC), dtype=np.int32)
        p = np.arange(128)
        for kc in range(KC):
            c4 = kc // FC
            if kc % FC == 0:
                gidx[:, kc] = (128 * c4 + p) * NJ + g
            else:
                gidx[:, kc] = (512 * (g // 2) + 128 * c4 + p) * 2 + (g % 2)
        in_maps.append(
            {
                "xT": _rearr(np.ascontiguousarray(x[b].T)),
                "wq": _rearr(Wq[:, fsl]),
                "wk": _rearr(Wkv[:, fsl]),
                "wv": _rearr(Wkv[:, D + g * FPC : D + (g + 1) * FPC]),
                "wo": _rearr(Wo),
                "bo": np.ascontiguousarray(
                    np.broadcast_to(bo.reshape(1, D), (128, D))
                ).astype(np.float32),
                "wqc": np.ascontiguousarray(
                    norm_q_w[fsl].reshape(FC, 128).T
                ).astype(BF),
                "wkc": np.ascontiguousarray(
                    norm_k_w[fsl].reshape(FC, 128).T
                ).astype(BF),
                "cos_t": cos_t.astype(BF),
                "sin_t": sin_t.astype(BF),
                "psw_t": psw_np.astype(BF),
                "gidx": gidx,
            }
        )

    res = run_bass_kernel_spmd(nc, in_maps, list(range(CORES)), trace=_trace)
    out = np.empty((B, N, D), dtype=np.float32)
    for c in range(CORES):
        b, g = c // 4, c % 4
        out[b, g * 512 : (g + 1) * 512, :] = np.asarray(
            res.results[c]["out"]
        ).astype(np.float32)
    if _trace or _dbg:
        return out, res
    return out


# revision 43
# speedup vs baseline: 1.0914x; 1.0914x over previous
"""Distributed Bass kernel for nn_Attention (B=2, N=2048, D=1024, H=16, DH=64) on 8 trn2 cores.

Sharding: data-parallel over batch (cores 0-3 -> b=0, 4-7 -> b=1), tensor-parallel
over heads (4 heads / 256 inner features per core).  v3 design:
  all matmuls bf16 (weights/x pre-cast+rearranged on host, fp32 PSUM accumulate);
  q/k projections + rope first (rope combine split DVE / ACT-copy+GPSIMD-add),
  ssq AllReduce (4-core groups) overlapped with v projections;
  k-side RMSNorm scale folded into the softmax exp *scale* (per-key partition
  scale AP) so kT needs no normalization pass at all;
  softmax exp split across two engine lanes: ACT table exp and a DVE
  Schraudolph exp (affine int16 + bf16 bitcast, ~38% of tiles);
  softmax denominator via ones-row in v_aug, fast-approx reciprocal;
  per-head-pair AllGather over the 4-core batch group (non-shared output),
  output projection in two rounds (even feature chunks overlap the final
  AllGather, odd chunks after it).
Host assembles the (2, 2048, 1024) output from the 8 (512, 1024) shards.
"""
import os
import sys

for _p in ("/opt/trn_rl_repo", "/root/.axon_site/_ro/trn_rl_repo"):
    if os.path.isdir(_p) and _p not in sys.path:
        sys.path.insert(0, _p)

import numpy as np
import ml_dtypes
import concourse.bass as bass
import concourse.mybir as mybir
import concourse.tile as tile
from concourse import bacc
from concourse.bass_utils import run_bass_kernel_spmd

dt = mybir.dt
AF = mybir.ActivationFunctionType
ALU = mybir.AluOpType
F32, BF16, I32, I16 = dt.float32, dt.bfloat16, dt.int32, dt.int16
BF = ml_dtypes.bfloat16

B, N, D = 2, 2048, 1024
H, DH = 16, 64
HPC = 4            # heads per core
FPC = HPC * DH     # 256 inner features per core
KC = D // 128      # 8 contraction chunks
FC = FPC // 128    # 2 feature chunks per core
NJ = N // 512      # 4 quarter chunks
NT = N // 128      # 16 m-tiles
EPS = 1e-6
CORES = 8
GROUPS4 = [[0, 1, 2, 3], [4, 5, 6, 7]]
JORDER = [2, 3, 0, 1]  # n-half 1 first: its ssq AllReduce fires early

L2E = float(np.log2(np.e))
SCH_SCALE = 128.0 * L2E            # Schraudolph bf16 exp scale
SCH_BIAS = 127.0 * 128.0 - 5.61    # fitted for RNE rounding, max rel ~3.3%

_CACHED_NC = None


def build(dbg=False):
    nc = bacc.Bacc("TRN2", target_bir_lowering=False, debug=False, num_devices=CORES)

    xT = nc.dram_tensor("xT", [128, KC, N], BF16, kind="ExternalInput")
    wq_d = nc.dram_tensor("wq", [128, KC, FPC], BF16, kind="ExternalInput")
    wk_d = nc.dram_tensor("wk", [128, KC, FPC], BF16, kind="ExternalInput")
    wv_d = nc.dram_tensor("wv", [128, KC, FPC], BF16, kind="ExternalInput")
    wo_d = nc.dram_tensor("wo", [128, KC, D], BF16, kind="ExternalInput")
    bo_d = nc.dram_tensor("bo", [128, D], F32, kind="ExternalInput")
    wqc_d = nc.dram_tensor("wqc", [128, FC], BF16, kind="ExternalInput")
    wkc_d = nc.dram_tensor("wkc", [128, FC], BF16, kind="ExternalInput")
    cos_d = nc.dram_tensor("cos_t", [128, N], BF16, kind="ExternalInput")
    sin_d = nc.dram_tensor("sin_t", [128, N], BF16, kind="ExternalInput")
    psw_d = nc.dram_tensor("psw_t", [128, 128], BF16, kind="ExternalInput")
    gidx_d = nc.dram_tensor("gidx", [128, KC], I32, kind="ExternalInput")
    out_d = nc.dram_tensor("out", [512, D], F32, kind="ExternalOutput")

    if dbg:
        dbg_qn = nc.dram_tensor("dbg_qn", [128, FC, N], BF16, kind="ExternalOutput")
        dbg_kt = nc.dram_tensor("dbg_kt", [128, FC, N], BF16, kind="ExternalOutput")
        dbg_rk = nc.dram_tensor("dbg_rk", [128, NT], F32, kind="ExternalOutput")
        dbg_osb = nc.dram_tensor("dbg_osb", [128, FC, N], BF16, kind="ExternalOutput")
        dbg_og = nc.dram_tensor("dbg_og", [128, KC, 512], BF16, kind="ExternalOutput")

    # collective bounce buffers
    rk_bounce = nc.dram_tensor("rk_bounce", [1, N], F32)
    ssq_in = [nc.dram_tensor(f"ssq_in{i}", [2, N // 2], F32) for i in range(2)]
    ssq_out = [nc.dram_tensor(f"ssq_out{i}", [2, N // 2], F32) for i in range(2)]
    ag_in0 = nc.dram_tensor("ag_in0", [128, N], BF16)
    ag_out0 = nc.dram_tensor("ag_out0", [4 * 128, N], BF16)
    # pair 1 gathers per query half; halves are contiguous row blocks
    ag_in1 = nc.dram_tensor("ag_in1", [2 * 128, N // 2], BF16)
    ag_out1 = nc.dram_tensor("ag_out1", [2 * 4 * 128, N // 2], BF16)

    with tile.TileContext(nc) as tc:
        with tc.tile_pool(name="persist", bufs=1) as pp:
            # ---- constants ------------------------------------------------
            ones_col32 = pp.tile([128, 1], F32, tag="onesc32")
            nc.gpsimd.memset(ones_col32[:], 1.0)
            ones_col_bf = pp.tile([128, 1], BF16, tag="onescbf")
            nc.vector.tensor_copy(ones_col_bf[:], ones_col32[:])
            ones_row32 = pp.tile([1, 128], F32, tag="onesr32")
            nc.gpsimd.memset(ones_row32[:], 1.0)
            ones_row_bf = pp.tile([1, 128], BF16, tag="onesrbf")
            nc.vector.tensor_copy(ones_row_bf[:], ones_row32[:])
            # activation bias values at consumer base partitions
            eps_t = pp.tile([33, 1], F32, tag="eps")
            nc.gpsimd.memset(eps_t[:], EPS)
            bexp_t = pp.tile([33, 1], F32, tag="bexp")
            nc.gpsimd.memset(bexp_t[:], 0.0)
            nc.gpsimd.memset(bexp_t[32:33, :], -float(np.log(8.0)))

            wqc_sb = pp.tile([128, FC], BF16, tag="wqc")
            wkc_sb = pp.tile([128, FC], BF16, tag="wkc")
            gidx_sb = pp.tile([128, KC], I32, tag="gidx")

            # ---- big persistent tensors ----------------------------------
            # DMA order matters: the first q/k matmul group needs xsb j=0 and
            # wq only, so those go first on the queue
            xsb = pp.tile([128, KC, N], BF16, tag="xsb")
            wq_sb = pp.tile([128, KC, FPC], BF16, tag="wq")
            wk_sb = pp.tile([128, KC, FPC], BF16, tag="wk")
            wv_sb = pp.tile([128, KC, FPC], BF16, tag="wv")
            cos_sb = pp.tile([128, N], BF16, tag="cos")
            sin_sb = pp.tile([128, N], BF16, tag="sin")
            # PE warmup: dummy matmuls from kernel start until the first real
            # projection inputs land -- moves the HAM to K=8/8 before phase A
            # and avoids a cold first half
            wrm = pp.tile([128, 512], BF16, tag="wrm")
            nc.gpsimd.memset(wrm[:], 1.0)
            with tc.tile_pool(name="psW", bufs=1, space="PSUM") as psW:
                wps = psW.tile([128, 512], F32, tag="wps")
                for _ in range(20):
                    nc.tensor.matmul(
                        wps[:], wrm[:, 0:128], wrm[:], start=True, stop=True
                    )

            # n-half 1 first (its ssq AllReduce fires early); initial loads
            # split across the sync/scalar queues so the first matmul group's
            # inputs land in ~6us instead of ~14us
            h1sl = slice(1024, 2048)
            h0sl = slice(0, 1024)
            nc.scalar.dma_start(out=wq_sb[:], in_=wq_d[:])
            for kc in range(KC):
                if kc < 6:
                    nc.sync.dma_start(out=xsb[:, kc, h1sl], in_=xT[:, kc, h1sl])
                else:
                    nc.scalar.dma_start(out=xsb[:, kc, h1sl], in_=xT[:, kc, h1sl])
            # norm-weight columns feed the very first rope STT: keep them at
            # the front of the (otherwise free) ACT queue
            nc.scalar.dma_start(out=wqc_sb[:], in_=wqc_d[:])
            nc.scalar.dma_start(out=wkc_sb[:], in_=wkc_d[:])
            # pair-swap permutation matrix (host-precomputed):
            # psw[p, 2f+e] = 1 iff p == 2f+1-e
            psw = pp.tile([128, 128], BF16, tag="psw")
            nc.sync.dma_start(out=psw[:], in_=psw_d[:])
            nc.sync.dma_start(out=cos_sb[:, h1sl], in_=cos_d[:, h1sl])
            nc.sync.dma_start(out=sin_sb[:, h1sl], in_=sin_d[:, h1sl])
            nc.sync.dma_start(out=wk_sb[:], in_=wk_d[:])
            nc.sync.dma_start(out=xsb[:, :, h0sl], in_=xT[:, :, h0sl])
            nc.sync.dma_start(out=cos_sb[:, h0sl], in_=cos_d[:, h0sl])
            nc.sync.dma_start(out=sin_sb[:, h0sl], in_=sin_d[:, h0sl])
            nc.sync.dma_start(out=gidx_sb[:], in_=gidx_d[:])
            # big late-use weights go on the gpsimd SWDGE queue, keeping the
            # ACT engine queue free for phase A compute
            nc.gpsimd.dma_start(out=wv_sb[:], in_=wv_d[:])
            wo_sb = pp.tile([128, KC, D], BF16, tag="wo")
            nc.gpsimd.dma_start(out=wo_sb[:], in_=wo_d[:])
            bo_sb = pp.tile([128, D], F32, tag="bo")
            nc.gpsimd.dma_start(out=bo_sb[:], in_=bo_d[:])
            # PE warmup: dummy matmuls from kernel start until the first real
            # projection inputs land -- moves the HAM to K=8/8 before phase A
            # and avoids a cold first half
            wrm = pp.tile([128, 512], BF16, tag="wrm")
            nc.gpsimd.memset(wrm[:], 1.0)
            with tc.tile_pool(name="psW", bufs=1, space="PSUM") as psW:
                wps = psW.tile([128, 512], F32, tag="wps")
                for _ in range(40):
                    nc.tensor.matmul(
                        wps[:], wrm[:, 0:128], wrm[:], start=True, stop=True
                    )

            kT = pp.tile([128, FC, N], BF16, tag="kT")
            qn = pp.tile([128, FC, N], BF16, tag="qn")
            v_aug = pp.tile([128, NT, HPC, DH + 1], BF16, tag="vaug")
            nc.vector.tensor_copy(
                v_aug[:, :, :, DH : DH + 1],
                ones_col32[:].to_broadcast([128, NT, HPC, 1]),
            )
            o_sb = pp.tile([128, FC, N], BF16, tag="osb")
            og = pp.tile([128, KC, 512], BF16, tag="og")
            rinv = pp.tile([1, N], BF16, tag="rinv")
            rk0 = pp.tile([1, N], F32, tag="rk0")
            rk8T = pp.tile([128, NT], F32, tag="rk8T")
            rk8Td = pp.tile([128, NT], F32, tag="rk8Td")

            # ================= PHASE A: q/k projections + rope =============
            with (
                tc.tile_pool(name="pa", bufs=1) as pa,
                tc.tile_pool(name="pwa", bufs=4) as pwa,
            ):
                qpre = pa.tile([128, FC, N], BF16, tag="qpre")
                # row-vector stripes at base partitions 0 (q) and 32 (k)
                ssqp = pa.tile([33, N], F32, tag="ssqp")
                sq2 = pa.tile([33, N], F32, tag="sq2")
                lnv = pa.tile([33, N], F32, tag="lnv")
                # rows 1-31 are never written by the ssq path but are read by
                # the combined [33, N] ln below; keep them finite
                nc.gpsimd.memset(sq2[:], 1.0)

                mul = ALU.mult
                with (
                    tc.tile_pool(name="psA", bufs=2, space="PSUM") as psA,
                    tc.tile_pool(name="psSw", bufs=1, space="PSUM") as psSw,
                    tc.tile_pool(name="psS", bufs=1, space="PSUM") as psS,
                ):
                  # 1024-wide tiles per (n-half, q/k, feature chunk): fewer,
                  # fatter elementwise ops keep the DVE/ACT queues short
                  for i in (1, 0):
                    hsl = slice(i * 1024, (i + 1) * 1024)
                    for ti, (w_sb, wcol, dest) in enumerate((
                        (wq_sb, wqc_sb, qpre),
                        (wk_sb, wkc_sb, kT),
                    )):
                        ssq_ps = psS.tile([1, 1024], F32, tag="ssq", name=f"ssq{i}{ti}")
                        for fc in range(FC):
                            fsl = slice(fc * 128, (fc + 1) * 128)
                            prj = psA.tile(
                                [128, 1024], F32, tag="proj", name=f"prj{i}{ti}{fc}"
                            )
                            for nh in range(2):
                                nsl = slice(nh * 512, nh * 512 + 512)
                                jsl = slice(i * 1024 + nh * 512, i * 1024 + nh * 512 + 512)
                                for kc in range(KC):
                                    nc.tensor.matmul(
                                        prj[:, nsl],
                                        w_sb[:, kc, fsl],
                                        xsb[:, kc, jsl],
                                        start=(kc == 0),
                                        stop=(kc == KC - 1),
                                    )
                            # sum-of-squares partial on ACT
                            q2 = pwa.tile([128, 1024], BF16, tag="q2", name=f"q2_{i}{ti}{fc}")
                            nc.scalar.activation(q2[:], prj[:], AF.Square)
                            for nh in range(2):
                                nsl = slice(nh * 512, nh * 512 + 512)
                                nc.tensor.matmul(
                                    ssq_ps[:, nsl],
                                    ones_col_bf[:],
                                    q2[:, nsl],
                                    start=(fc == 0),
                                    stop=(fc == FC - 1),
                                )
                            # rope with norm weight folded in; reads prj PSUM
                            tcos = pwa.tile([128, 1024], BF16, tag="tcos", name=f"tc{i}{ti}{fc}")
                            nc.vector.scalar_tensor_tensor(
                                tcos[:], prj[:], wcol[:, fc : fc + 1], cos_sb[:, hsl],
                                op0=mul, op1=mul,
                            )
                            tsin = pwa.tile([128, 1024], BF16, tag="tsin", name=f"ts{i}{ti}{fc}")
                            nc.vector.scalar_tensor_tensor(
                                tsin[:], prj[:], wcol[:, fc : fc + 1], sin_sb[:, hsl],
                                op0=mul, op1=mul,
                            )
                            swp = psSw.tile([128, 1024], F32, tag="swp", name=f"sw{i}{ti}{fc}")
                            for nh in range(2):
                                nsl = slice(nh * 512, nh * 512 + 512)
                                nc.tensor.matmul(
                                    swp[:, nsl], psw[:], tsin[:, nsl],
                                    start=True, stop=True,
                                )
                            # rope combine on DVE (gpsimd must stay free so
                            # the ssq AllReduce triggers fire promptly)
                            nc.vector.tensor_add(dest[:, fc, hsl], tcos[:], swp[:])
                        nc.scalar.activation(
                            ssqp[32 * ti : 32 * ti + 1, hsl], ssq_ps[:], AF.Copy
                        )
                    # fire this n-half's ssq AllReduce; bounce DMAs ride the
                    # near-empty ACT queue so the trigger is prompt
                    nc.scalar.dma_start(out=ssq_in[i][0:1, :], in_=ssqp[0:1, hsl])
                    nc.scalar.dma_start(out=ssq_in[i][1:2, :], in_=ssqp[32:33, hsl])
                    nc.gpsimd.collective_compute(
                        "AllReduce",
                        ALU.add,
                        replica_groups=GROUPS4,
                        ins=[ssq_in[i][:]],
                        outs=[ssq_out[i][:]],
                    )
                    nc.sync.dma_start(out=sq2[0:1, hsl], in_=ssq_out[i][0:1, :])
                    nc.sync.dma_start(out=sq2[32:33, hsl], in_=ssq_out[i][1:2, :])

                # ---- v projections part 1 (overlap the AllReduce tail) ----
                with (
                    tc.tile_pool(name="psV", bufs=4, space="PSUM") as psV,
                    tc.tile_pool(name="psBC", bufs=2, space="PSUM") as psBC,
                ):
                    def v_proj(nt):
                        nsl = slice((nt % 8) * 128, (nt % 8 + 1) * 128)
                        jsl = slice((nt // 8) * 1024, (nt // 8) * 1024 + 1024)
                        vps = psV.tile([128, FPC], F32, tag="v", name=f"v{nt}")
                        for kc in range(KC):
                            nc.tensor.matmul(
                                vps[:],
                                xsb[:, kc, jsl][:, nsl],
                                wv_sb[:, kc, :],
                                start=(kc == 0),
                                stop=(kc == KC - 1),
                            )
                        nc.scalar.activation(
                            v_aug[:, nt, :, 0:DH],
                            vps[:].rearrange("p (h f) -> p h f", f=DH),
                            AF.Copy,
                        )

                    for nt in range(12):
                        v_proj(nt)

                    # rq = rsqrt(ssq/D + eps) = exp(-0.5*ln(ssq/D + eps))
                    # rk8 = rq_k / 8      (score scale folded in, bias=-ln 8)
                    for i in (1, 0):
                        hsl = slice(i * 1024, (i + 1) * 1024)
                        nc.scalar.activation(
                            lnv[:, hsl], sq2[:, hsl], AF.Ln, scale=1.0 / D,
                            bias=eps_t[:],
                        )
                        nc.scalar.activation(
                            rinv[0:1, hsl], lnv[0:1, hsl], AF.Exp, scale=-0.5,
                        )
                        nc.scalar.activation(
                            rk0[0:1, hsl], lnv[32:33, hsl], AF.Exp, scale=-0.5,
                            bias=bexp_t[32:33, :],
                        )

                    # qn = qpre * broadcast(rq); k needs no scaling (rk8 is
                    # folded into the per-key exp scale APs below)
                    bq = {}
                    for i in (1, 0):
                        bq[i] = psBC.tile([128, 1024], F32, tag="bq", name=f"bq{i}")
                        for q2_ in range(2):
                            q2sl = slice(q2_ * 512, q2_ * 512 + 512)
                            hq2 = slice(i * 1024 + q2_ * 512, i * 1024 + q2_ * 512 + 512)
                            nc.tensor.matmul(
                                bq[i][:, q2sl], ones_row_bf[:], rinv[0:1, hq2],
                                start=True, stop=True,
                            )

                    for nt in range(12, NT):
                        v_proj(nt)

                    for i in (1, 0):
                        hsl = slice(i * 1024, (i + 1) * 1024)
                        for fc in range(FC):
                            nc.vector.tensor_mul(
                                qn[:, fc, hsl], qpre[:, fc, hsl], bq[i][:]
                            )

                    # per-key exp scales: rk8T[p, mt] = rk0[mt*128+p],
                    # rk8Td = rk8T * 128*log2(e)  (Schraudolph lane).
                    # Transposed via a DRAM bounce (both DMAs on the same
                    # in-order queue) -- a cross-partition rearrange read of
                    # SBUF is not dependency-tracked reliably.
                    nc.sync.dma_start(out=rk_bounce[:], in_=rk0[:])
                    nc.sync.dma_start(
                        out=rk8T[:],
                        in_=rk_bounce[0:1, :].rearrange("r (m p) -> (r p) m", p=128),
                    )
                    nc.vector.tensor_scalar(
                        rk8Td[:], rk8T[:], SCH_SCALE, None, op0=mul,
                    )

            # ============== PHASE B: attention, per head pair ==============
            expctr = 0
            with (
                tc.tile_pool(name="pwb", bufs=2) as pwb,
                tc.tile_pool(name="ppb", bufs=4) as ppb,
                tc.tile_pool(name="psB", bufs=2, space="PSUM") as psB,
                tc.tile_pool(name="psOV", bufs=1, space="PSUM") as psOV,
            ):
                # units interleave the two heads so consecutive ov matmuls
                # accumulate into different psum banks (back-to-back same-bank
                # accumulation serializes on the bank's read-modify-write)
                for ch in range(FC):
                    for hf in (1, 0):
                        # four independent 1-bank accumulators (head x query
                        # quarter); rows 0-63 = o, 64 = denominator
                        ov = {
                            (h2, j2): psOV.tile(
                                [DH + 1, 512], F32, tag=f"ov{h2}{j2}",
                                name=f"ov{ch}{hf}{h2}{j2}",
                            )
                            for h2 in range(2)
                            for j2 in range(2)
                        }
                        units = [(j2, mt) for j2 in range(2) for mt in range(NT)]

                        def emit_scores(k):
                            j2, mt = units[k]
                            jsl = slice(
                                hf * 1024 + j2 * 512, hf * 1024 + j2 * 512 + 512
                            )
                            s = {}
                            for h2 in range(2):
                                po = 64 * h2
                                st = psB.tile(
                                    [128, 512], F32, tag=f"s{h2}",
                                    name=f"s{ch}{hf}{j2}{mt}{h2}",
                                )
                                for mh in range(2):
                                    msl = slice(
                                        mt * 128 + mh * 64, mt * 128 + mh * 64 + 64
                                    )
                                    nc.tensor.matmul(
                                        st[mh * 64 : (mh + 1) * 64, :],
                                        kT[po : po + 64, ch, msl],
                                        qn[po : po + 64, ch, jsl],
                                        start=True,
                                        stop=True,
                                    )
                                s[h2] = st
                            return s

                        def emit_expov(k, s):
                            nonlocal expctr
                            j2, mt = units[k]
                            # DVE exps kept away from block boundaries where
                            # the DVE queue carries the evict chains
                            dve_ok = 3 <= k < 29
                            for h2 in range(2):
                                h = 2 * ch + h2
                                p = ppb.tile(
                                    [128, 512], BF16, tag=f"p{h2}",
                                    name=f"p{ch}{hf}{j2}{mt}{h2}",
                                )
                                if dve_ok and (expctr * 3) % 8 < 3:
                                    # Schraudolph exp on DVE (int16 + bitcast)
                                    nc.vector.tensor_scalar(
                                        p[:].bitcast(I16), s[h2][:],
                                        rk8Td[:, mt : mt + 1], SCH_BIAS,
                                        op0=ALU.mult, op1=ALU.add,
                                    )
                                else:
                                    nc.scalar.activation(
                                        p[:], s[h2][:], AF.Exp,
                                        scale=rk8T[:, mt : mt + 1],
                                    )
                                expctr += 1
                                nc.tensor.matmul(
                                    ov[(h2, j2)][0 : DH + 1, :],
                                    v_aug[:, mt, h, :],
                                    p[:],
                                    start=(mt == 0),
                                    stop=(mt == NT - 1),
                                )
                            if mt == NT - 1:
                                # this j2 quarter's accumulation is complete
                                # for both heads: evict while the rest runs
                                for h2 in range(2):
                                    emit_evict(j2, h2)

                        def emit_evict(j2, h2):
                            po = 64 * h2
                            ovt = ov[(h2, j2)]
                            # denominator row: psum partition 64 -> sbuf
                            # partition 0 (partition-shifting copy)
                            den = pwb.tile(
                                [1, 512], F32, tag=f"den{h2}",
                                name=f"den{ch}{hf}{j2}{h2}",
                            )
                            nc.vector.tensor_copy(den[:], ovt[DH : DH + 1, :])
                            rec = pwb.tile(
                                [1, 512], F32, tag=f"rec{h2}",
                                name=f"rec{ch}{hf}{j2}{h2}",
                            )
                            nc.vector.reciprocal_approx_fast(rec[:], den[:])
                            bc = pwb.tile(
                                [DH, 512], F32, tag=f"bc{h2}",
                                name=f"bc{ch}{hf}{j2}{h2}",
                            )
                            nc.gpsimd.partition_broadcast(bc[:], rec[:])
                            nc.vector.tensor_mul(
                                o_sb[po : po + 64, ch,
                                     hf * 1024 + j2 * 512 : hf * 1024 + j2 * 512 + 512],
                                ovt[0:DH, :],
                                bc[:],
                            )

                        prev = None
                        for k in range(len(units)):
                            cur = emit_scores(k)
                            if prev is not None:
                                emit_expov(k - 1, prev)
                            prev = cur
                        emit_expov(len(units) - 1, prev)

                        if ch == 1:
                            # pair 1 gathers per query half so only the last
                            # (half-size) AllGather is exposed at the tail
                            hsl = slice(hf * 1024, hf * 1024 + 1024)
                            rsl = slice(hf * 128, hf * 128 + 128)
                            nc.sync.dma_start(
                                out=ag_in1[rsl, :], in_=o_sb[:, 1, hsl]
                            )
                            nc.gpsimd.collective_compute(
                                "AllGather",
                                ALU.bypass,
                                replica_groups=GROUPS4,
                                ins=[ag_in1[rsl, :]],
                                outs=[ag_out1[hf * 512 : hf * 512 + 512, :]],
                            )

                    if ch == 0:
                        # pair 0: one full gather, hidden under pair 1 compute
                        nc.sync.dma_start(out=ag_in0[:], in_=o_sb[:, 0, :])
                        nc.gpsimd.collective_compute(
                            "AllGather",
                            ALU.bypass,
                            replica_groups=GROUPS4,
                            ins=[ag_in0[:]],
                            outs=[ag_out0[:]],
                        )

                # og gathers: ch0 rows are ready once AG0 lands (mid pair 1);
                # each core picks its own n-quarter via the gidx input
                for kc in range(0, KC, FC):
                    nc.gpsimd.indirect_dma_start(
                        out=og[:, kc, :],
                        out_offset=None,
                        in_=ag_out0[:].rearrange("r (j n) -> (r j) n", n=512),
                        in_offset=bass.IndirectOffsetOnAxis(
                            ap=gidx_sb[:, kc : kc + 1], axis=0
                        ),
                    )
                for kc in range(1, KC, FC):
                    nc.gpsimd.indirect_dma_start(
                        out=og[:, kc, :],
                        out_offset=None,
                        in_=ag_out1[:].rearrange("r (q n) -> (r q) n", n=512),
                        in_offset=bass.IndirectOffsetOnAxis(
                            ap=gidx_sb[:, kc : kc + 1], axis=0
                        ),
                    )

            if dbg:
                nc.gpsimd.dma_start(out=dbg_qn[:], in_=qn[:])
                nc.gpsimd.dma_start(out=dbg_kt[:], in_=kT[:])
                nc.sync.dma_start(out=dbg_rk[:], in_=rk8T[:])
                nc.sync.dma_start(out=dbg_osb[:], in_=o_sb[:])
                nc.sync.dma_start(out=dbg_og[:], in_=og[:])

            # ========= PHASE C: output projection ==========================
            # round 1: even contraction chunks (need only AG0) overlap AG1;
            # round 2: odd chunks + evictions after AG1 lands
            with (
                tc.tile_pool(name="pc", bufs=2) as pc,
                tc.tile_pool(name="psC", bufs=1, space="PSUM") as psC,
            ):
                yps = {}
                for ntl in range(4):
                    yps[ntl] = psC.tile(
                        [128, D], F32, tag=f"y{ntl}", name=f"yps{ntl}"
                    )
                    for dc in range(2):
                        dsl = slice(dc * 512, (dc + 1) * 512)
                        for i, kc in enumerate(range(0, KC, 2)):
                            nc.tensor.matmul(
                                yps[ntl][:, dsl],
                                og[:, kc, ntl * 128 : (ntl + 1) * 128],
                                wo_sb[:, kc, dsl],
                                start=(i == 0),
                                stop=False,
                            )
                for ntl in range(4):
                    for dc in range(2):
                        dsl = slice(dc * 512, (dc + 1) * 512)
                        for i, kc in enumerate(range(1, KC, 2)):
                            nc.tensor.matmul(
                                yps[ntl][:, dsl],
                                og[:, kc, ntl * 128 : (ntl + 1) * 128],
                                wo_sb[:, kc, dsl],
                                start=False,
                                stop=(i == KC // 2 - 1),
                            )
                    # bias folded into the eviction (host pre-broadcast to all
                    # 128 partitions)
                    ysb = pc.tile([128, D], F32, tag="ysb", name=f"ysb{ntl}")
                    nc.vector.tensor_add(ysb[:], yps[ntl][:], bo_sb[:])
                    nc.sync.dma_start(
                        out=out_d[ntl * 128 : (ntl + 1) * 128, :], in_=ysb[:]
                    )

    nc.compile()
    return nc


def _rope_tables():
    """cos/sin tables matching the reference's f32 angle computation.

    C[d, n] = cos(n * theta[d//2]);  Ssw[2i] = +sin, Ssw[2i+1] = -sin
    (Ssw is the swapped-operand multiplier: rope = x*C + swap(x*Ssw)).
    Tiled x2 along partitions to cover a 2-head (128-row) chunk.
    """
    i2 = np.arange(0, DH, 2, dtype=np.float32)
    theta = (1.0 / (10000.0 ** (i2 / DH))).astype(np.float32)  # (32,)
    ang = np.arange(N, dtype=np.float32)[:, None] * theta[None, :]  # (N, 32) f32
    cos = np.cos(ang.astype(np.float64)).astype(np.float32).T  # (32, N)
    sin = np.sin(ang.astype(np.float64)).astype(np.float32).T
    cos_d = np.repeat(cos, 2, axis=0)  # (64, N)
    ssw = np.repeat(sin, 2, axis=0)
    ssw[1::2, :] *= -1.0
    cos_t = np.tile(cos_d, (2, 1)).astype(np.float32)  # (128, N)
    sin_t = np.tile(ssw, (2, 1)).astype(np.float32)
    return cos_t, sin_t


def _rearr(w):
    # [D, F] -> [128, KC, F] grouping the contraction dim into 128-row chunks
    d, f = w.shape
    return np.ascontiguousarray(
        w.reshape(KC, 128, f).transpose(1, 0, 2).astype(BF)
    )


def kernel(x, Wq, Wkv, norm_q_w, norm_k_w, Wo, bo, _trace=False, _dbg=False):
    global _CACHED_NC
    x = np.asarray(x, dtype=np.float32)
    Wq = np.asarray(Wq, dtype=np.float32)
    Wkv = np.asarray(Wkv, dtype=np.float32)
    norm_q_w = np.asarray(norm_q_w, dtype=np.float32)
    norm_k_w = np.asarray(norm_k_w, dtype=np.float32)
    Wo = np.asarray(Wo, dtype=np.float32)
    bo = np.asarray(bo, dtype=np.float32)

    cos_t, sin_t = _rope_tables()
    psw_np = np.zeros((128, 128), dtype=np.float32)
    pidx = np.arange(128)
    psw_np[pidx ^ 1, pidx] = 1.0  # column 2f+e reads row 2f+1-e
    if _dbg:
        nc = build(dbg=True)
    else:
        if _CACHED_NC is None:
            _CACHED_NC = build()
        nc = _CACHED_NC

    in_maps = []
    for c in range(CORES):
        b, g = c // 4, c % 4
        fsl = slice(g * FPC, (g + 1) * FPC)
        # even kc (pair 0): ag_out0 [512, 2048] -> rows (128*c + p)*4 + g
        # odd kc (pair 1): ag_out1 [1024, 1024] half-major
        #   -> rows (512*(g//2) + 128*c + p)*2 + (g%2)
        gidx = np.empty((128, KC), dtype=np.int32)
        p = np.arange(128)
        for kc in range(KC):
            c4 = kc // FC
            if kc % FC == 0:
                gidx[:, kc] = (128 * c4 + p) * NJ + g
            else:
                gidx[:, kc] = (512 * (g // 2) + 128 * c4 + p) * 2 + (g % 2)
        in_maps.append(
            {
                "xT": _rearr(np.ascontiguousarray(x[b].T)),
                "wq": _rearr(Wq[:, fsl]),
                "wk": _rearr(Wkv[:, fsl]),
                "wv": _rearr(Wkv[:, D + g * FPC : D + (g + 1) * FPC]),
                "wo": _rearr(Wo),
                "bo": np.ascontiguousarray(
                    np.broadcast_to(bo.reshape(1, D), (128, D))
                ).astype(np.float32),
                "wqc": np.ascontiguousarray(
                    norm_q_w[fsl].reshape(FC, 128).T
                ).astype(BF),
                "wkc": np.ascontiguousarray(
                    norm_k_w[fsl].reshape(FC, 128).T
                ).astype(BF),
                "cos_t": cos_t.astype(BF),
                "sin_t": sin_t.astype(BF),
                "psw_t": psw_np.astype(BF),
                "gidx": gidx,
            }
        )

    res = run_bass_kernel_spmd(nc, in_maps, list(range(CORES)), trace=_trace)
    out = np.empty((B, N, D), dtype=np.float32)
    for c in range(CORES):
        b, g = c // 4, c % 4
        out[b, g * 512 : (g + 1) * 512, :] = np.asarray(
            res.results[c]["out"]
        ).astype(np.float32)
    if _trace or _dbg:
        return out, res
    return out
